# revision 1
# baseline (speedup 1.0000x reference)
# Contextual loss kernel for Trainium2, 8 NeuronCores.
#
# Reference computation (see problem):
#   y_mu = mean(y, axis=(0,2,3))                       # per channel
#   xn = normalize(x - y_mu, axis=C); yn = normalize(y - y_mu, axis=C)
#   A[n,p,q] = sum_c xn[n,c,p] * yn[n,c,q]             # cosine similarity
#   dist = 1 - A;  dist_tilde = dist / (min_q dist + EPS)
#   w = exp((1 - dist_tilde)/bw);  cx = w / sum_q w
#   loss = mean_n(-log(mean_q max_p cx + EPS))
#
# Sharding: core c handles sample n=c//2, row-half h=c%2 (2048 of the 4096
# p-rows). Each core returns the per-column partial max m_q of cx over its
# rows; the host combines the two halves per sample (elementwise max), takes
# the mean over q and the -log/mean over samples (O(8*4096) scalar work).
#
# Per-core pipeline (per 128-row block):
#   PE    : A-half-block [128,2048] into PSUM (bf16 inputs, fp32 accum)
#   DVE   : tensor_tensor_reduce = evacuate PSUM -> SBUF bf16, folding the
#           1/||xc_p|| row scale in via a broadcast operand, and computing
#           the row max as the fused reduction
#   DVE   : tiny [128,1] chain -> per-row scale t and bias b for the exp
#   ACT   : w = Exp(t*A + b) with fused row-sum accumulator S
#   DVE   : scalar_tensor_tensor: M = max(w * (1/S), M)   (column max acc.)
# Final fold of M [128,4096] across partitions via PE transpose + DVE
# reduce_max -> m[4096].
#
# x is NOT normalized on the input side: A = (xc^T yn) * (1/||xc_p||) and the
# row scale folds into the PSUM evacuation / exp scale for free.

import numpy as np

N, C, H, W = 4, 256, 64, 64
P = H * W            # 4096
HALF = P // 2        # 2048
NCORES = 8
BW = 0.5
EPS = 1e-5
NEG_INIT = -1.0e30

_cache = {}
_PHASES = 3  # test hook: 1=prep only, 2=+main loop, 3=full


def _patched_tile_context(tile_mod, nc):
    """TileContext whose tail drain splits its sem waits one-per-drain.

    The walrus build in this container rejects a Drain instruction carrying
    more than one sync wait ("Too many sync wait commands"), and the stock
    TileContext attaches the whole global clock to a single drain.
    """
    from concourse.vector_clock import ScopedClock

    class TC(tile_mod.TileContext):
        def _drain_and_barrier(self, tick_clock, wait_clock):
            nc_ = self.nc
            drain_inst = nc_.sync.drain()
            wait_clock.add_sem_waits(
                drain_inst.ins, ScopedClock({None: tick_clock.global_clock})
            )
            si = drain_inst.ins.sync_info
            waits = list(si.on_wait or []) if si is not None else []
            if len(waits) > 1:
                si.on_wait = waits[:1]
                rest = waits[1:]
                while rest:
                    d2 = nc_.sync.drain()
                    if d2.ins.sync_info is None:
                        d2.ins.sync_info = type(si)(on_wait=rest[:1], on_update=[])
                    else:
                        d2.ins.sync_info.on_wait = rest[:1]
                    rest = rest[1:]
            nc_.all_engine_barrier()
            assert self.sems is not None
            popped = nc_._tile_sem_poison_stack.pop()
            assert popped is self._sem_poison
            nc_.clear_and_free_semaphores(list(self.sems.allocated().values()))
            nc_.all_engine_barrier()

    return TC(nc)


def _split_excess_waits(nc, mybir, maxw=1):
    """Hoist sync waits beyond `maxw` per instruction onto EventSemaphore
    carrier instructions inserted just before, on the same engine.

    This walrus build rejects instructions carrying more than ~2 sync
    waits ("Too many sync wait commands"); Tile attaches up to ~10.
    Executing the waits on earlier same-engine instructions preserves the
    happens-before semantics exactly.
    """
    k = 0
    for fn in nc.m.functions:
        for blk in fn.blocks:
            il = blk.instructions
            new = []
            changed = False
            for ins in il:
                si = getattr(ins, "sync_info", None)
                waits = list(si.on_wait) if (si is not None and si.on_wait) else []
                if len(waits) > maxw:
                    changed = True
                    extra, keep = waits[:-maxw], waits[-maxw:]
                    while extra:
                        chunk, extra = extra[:maxw], extra[maxw:]
                        ev = mybir.InstEventSemaphore(name=f"I-sw{k}")
                        k += 1
                        ev.engine = ins.engine
                        ev.sync_info = type(si)(on_wait=chunk, on_update=[])
                        new.append(ev)
                    si.on_wait = keep
                new.append(ins)
            if changed:
                blk.instructions = new


def _build_nc():
    from contextlib import ExitStack

    import concourse.bass as bass
    import concourse.tile as tile
    from concourse import mybir
    from concourse.masks import make_identity

    fp32 = mybir.dt.float32
    bf16 = mybir.dt.bfloat16
    X = mybir.AxisListType.X
    OP = mybir.AluOpType
    AF = mybir.ActivationFunctionType

    nc = bass.Bass("TRN2", target_bir_lowering=False)
    xh_d = nc.declare_dram_parameter("xh", [C, HALF], fp32, isOutput=False)
    yn_d = nc.declare_dram_parameter("yn", [C, P], fp32, isOutput=False)
    m_d = nc.declare_dram_parameter("m_out", [32, 128], fp32, isOutput=True)

    with _patched_tile_context(tile, nc) as tc, ExitStack() as ctx:
        const = ctx.enter_context(tc.tile_pool(name="const", bufs=1))
        persist = ctx.enter_context(tc.tile_pool(name="persist", bufs=1))
        dram = ctx.enter_context(tc.tile_pool(name="dram", bufs=1, space="DRAM"))

        ones_f = const.tile([128, 1], fp32)
        nc.vector.memset(ones_f, 1.0)
        ident = const.tile([128, 128], bf16)
        make_identity(nc, ident)

        # persistent tiles
        ynb = [persist.tile([128, P], bf16, tag=f"ynb{h}", name=f"ynb{h}") for h in range(2)]
        xnb = [persist.tile([128, HALF], bf16, tag=f"xnb{h}", name=f"xnb{h}") for h in range(2)]
        inx = persist.tile([128, 16], fp32, tag="inx")
        Macc = persist.tile([128, P], bf16, tag="Macc")
        mfold = persist.tile([128, 32], fp32, tag="mfold")
        negmu = persist.tile([128, 2], fp32, tag="negmu")

        # ---------------- phases 0+1 share the "prep" scope ----------------
        prep = ctx.enter_context(tc.tile_pool(name="prep", bufs=1))
        yc = [
            prep.tile([128, P], fp32, tag=f"yc{h}", name=f"yc{h}") for h in range(2)
        ]

        # ---------------- phase 0: y_mu via AllReduce ----------------
        # Each core reduces its own sample's y over the spatial dim; a 1 KB
        # AllReduce(add) across the 8 cores yields sum over (n, p) with each
        # sample counted twice (the two row-half cores share a sample).
        for h in range(2):
            nc.sync.dma_start(out=yc[h], in_=yn_d[h * 128 : (h + 1) * 128, :])
        with tc.tile_pool(name="ph0", bufs=1) as ph0:
            part2 = ph0.tile([128, 2], fp32)
            for h in range(2):
                nc.vector.tensor_reduce(
                    out=part2[:, h : h + 1], in_=yc[h], axis=X, op=OP.add
                )
            cc_in = dram.tile([128, 2], fp32, name="cc_in")
            cc_out = dram.tile([128, 2], fp32, name="cc_out")
            nc.sync.dma_start(out=cc_in[:, :], in_=part2)
            nc.gpsimd.collective_compute(
                "AllReduce",
                OP.add,
                replica_groups=[list(range(NCORES))],
                ins=[cc_in[:, :]],
                outs=[cc_out[:, :]],
            )
            allred = ph0.tile([128, 2], fp32)
            nc.sync.dma_start(out=allred, in_=cc_out[:, :])
            nc.vector.tensor_scalar_mul(
                out=negmu, in0=allred, scalar1=-1.0 / float(2 * N * P)
            )

        # ---------------- phase 1: center, norms, casts ----------------
        with tc.tile_pool(name="ph1", bufs=1) as ph1, tc.tile_pool(
            name="ph1ps", bufs=1, space="PSUM"
        ) as ph1ps, tc.tile_pool(name="ph1sm", bufs=1) as ph1sm:
            # ---- y side
            for h in range(2):
                nc.vector.tensor_scalar_add(
                    out=yc[h], in0=yc[h], scalar1=negmu[:, h : h + 1]
                )
            nrm_y = ph1ps.tile([1, P], fp32, tag="nrm")
            for h in range(2):
                ysq = ph1.tile([128, P], fp32, tag="ysq", name="ysq")
                nc.scalar.activation(out=ysq, in_=yc[h], func=AF.Square)
                for j in range(P // 512):
                    nc.tensor.matmul(
                        nrm_y[0:1, j * 512 : (j + 1) * 512],
                        lhsT=ones_f,
                        rhs=ysq[:, j * 512 : (j + 1) * 512],
                        start=(h == 0),
                        stop=(h == 1),
                    )
            # bounce [1,P] -> [128,32] (partition-major); DMA cannot read PSUM
            nrm_y_sb = ph1sm.tile([1, P], fp32, tag="nrm_y_sb")
            nc.scalar.copy(nrm_y_sb, nrm_y[0:1, :])
            dy = dram.tile([32, 128], fp32, tag="dy")
            nc.sync.dma_start(
                out=dy[:, :].rearrange("j p -> (j p)").rearrange("(a f) -> a f", a=1),
                in_=nrm_y_sb[0:1, :],
            )
            nsq_y = ph1sm.tile([128, 32], fp32, tag="nsq_y")
            nc.sync.dma_start(out=nsq_y, in_=dy[:, :].rearrange("j p -> p j"))
            iny = ph1sm.tile([128, 32], fp32, tag="iny")
            _inv_sqrt(nc, mybir, ph1sm, nsq_y, iny)
            # bounce back and broadcast to [128, P]
            dyb = dram.tile([32, 128], fp32, tag="dyb")
            nc.sync.dma_start(out=dyb[:, :].rearrange("j p -> p j"), in_=iny)
            inyb = ph1.tile([128, P], fp32, tag="inyb")
            src = bass.AP(
                tensor=dyb.tensor, offset=dyb.offset, ap=[[0, 128], [1, P]]
            )
            nc.sync.dma_start(out=inyb, in_=src)
            for h in range(2):
                nc.vector.tensor_mul(ynb[h], yc[h], inyb)

            # ---- x side
            xc = [ph1.tile([128, HALF], fp32, tag=f"xc{h}", name=f"xc{h}") for h in range(2)]
            for h in range(2):
                nc.sync.dma_start(out=xc[h], in_=xh_d[h * 128 : (h + 1) * 128, :])
                nc.vector.tensor_scalar_add(
                    out=xc[h], in0=xc[h], scalar1=negmu[:, h : h + 1]
                )
            nrm_x = ph1ps.tile([1, HALF], fp32, tag="nrm")
            for h in range(2):
                xsq = ph1.tile([128, HALF], fp32, tag="xsq", name="xsq")
                nc.scalar.activation(out=xsq, in_=xc[h], func=AF.Square)
                for j in range(HALF // 512):
                    nc.tensor.matmul(
                        nrm_x[0:1, j * 512 : (j + 1) * 512],
                        lhsT=ones_f,
                        rhs=xsq[:, j * 512 : (j + 1) * 512],
                        start=(h == 0),
                        stop=(h == 1),
                    )
            nrm_x_sb = ph1sm.tile([1, HALF], fp32, tag="nrm_x_sb")
            nc.scalar.copy(nrm_x_sb, nrm_x[0:1, :])
            dx = dram.tile([16, 128], fp32, tag="dx")
            nc.sync.dma_start(
                out=dx[:, :].rearrange("j p -> (j p)").rearrange("(a f) -> a f", a=1),
                in_=nrm_x_sb[0:1, :],
            )
            nsq_x = ph1sm.tile([128, 16], fp32, tag="nsq_x")
            nc.sync.dma_start(out=nsq_x, in_=dx[:, :].rearrange("j p -> p j"))
            _inv_sqrt(nc, mybir, ph1sm, nsq_x, inx)
            for h in range(2):
                nc.vector.tensor_copy(xnb[h], xc[h])

        # ---------------- phase 2: main loop (two passes) ----------------
        # Pass A: compute per-row maxima of A for every block; the PSUM tile's
        # only consumer is the reduce, so PE never stalls on the exp chain.
        # Then one batched [128,16] chain computes every block's exp
        # scale/bias. Pass B recomputes the matmuls (PE has headroom) and the
        # exp consumes PSUM directly, with the row-softmax + column-max fused
        # ops behind it.
        nc.vector.memset(Macc, 0.0)
        nblocks = (HALF // 128) if _PHASES >= 2 else 0
        rmall = persist.tile([128, 16], fp32, tag="rmall")
        tsc = persist.tile([128, 16], fp32, tag="tsc")
        bsc = persist.tile([128, 16], fp32, tag="bsc")
        with tc.tile_pool(name="mmpsA", bufs=2, space="PSUM") as mmpsA, tc.tile_pool(
            name="smA", bufs=3
        ) as smA:
            for r in range(nblocks):
                rm2 = smA.tile([128, 2], fp32, tag="rm2")
                for half in range(2):
                    ps = mmpsA.tile([128, HALF], fp32, tag="psA", name=f"psA{half}")
                    for h in range(2):
                        lhs = xnb[h][:, r * 128 : (r + 1) * 128]
                        for j in range(HALF // 512):
                            q0 = half * HALF + j * 512
                            nc.tensor.matmul(
                                ps[:, j * 512 : (j + 1) * 512],
                                lhsT=lhs,
                                rhs=ynb[h][:, q0 : q0 + 512],
                                start=(h == 0),
                                stop=(h == 1),
                            )
                    nc.vector.tensor_reduce(
                        out=rm2[:, half : half + 1], in_=ps, axis=X, op=OP.max
                    )
                nc.vector.tensor_reduce(
                    out=rmall[:, r : r + 1], in_=rm2, axis=X, op=OP.max
                )
        if _PHASES >= 2:
            # batched chain: rmax = rmall*inx; t = 1/(bw*(1-rmax+eps));
            # scale = t*inx; bias = t*(eps-rmax)
            with tc.tile_pool(name="chain", bufs=1) as chain:
                rmaxn = chain.tile([128, 16], fp32)
                nc.vector.tensor_mul(rmaxn, rmall, inx)
                bwd = chain.tile([128, 16], fp32)
                nc.vector.tensor_scalar(
                    out=bwd,
                    in0=rmaxn,
                    scalar1=-BW,
                    scalar2=BW * (1.0 + EPS),
                    op0=OP.mult,
                    op1=OP.add,
                )
                t_ = chain.tile([128, 16], fp32)
                nc.vector.reciprocal(t_, bwd)
                nc.vector.tensor_mul(tsc, t_, inx)
                e_ = chain.tile([128, 16], fp32)
                nc.vector.tensor_scalar(
                    out=e_, in0=rmaxn, scalar1=-1.0, scalar2=EPS, op0=OP.mult, op1=OP.add
                )
                nc.vector.tensor_mul(bsc, e_, t_)
        with tc.tile_pool(name="mmpsB", bufs=2, space="PSUM") as mmpsB, tc.tile_pool(
            name="wpool", bufs=2
        ) as wpool, tc.tile_pool(name="smB", bufs=3) as smB:
            for r in range(nblocks):
                pss = []
                for half in range(2):
                    ps = mmpsB.tile([128, HALF], fp32, tag="psB", name=f"psB{half}")
                    pss.append(ps)
                    for h in range(2):
                        lhs = xnb[h][:, r * 128 : (r + 1) * 128]
                        for j in range(HALF // 512):
                            q0 = half * HALF + j * 512
                            nc.tensor.matmul(
                                ps[:, j * 512 : (j + 1) * 512],
                                lhsT=lhs,
                                rhs=ynb[h][:, q0 : q0 + 512],
                                start=(h == 0),
                                stop=(h == 1),
                            )
                w_ = wpool.tile([128, P], bf16, tag="w")
                S2 = smB.tile([128, 2], fp32, tag="S2")
                for half in range(2):
                    nc.scalar.activation(
                        out=w_[:, half * HALF : (half + 1) * HALF],
                        in_=pss[half],
                        func=AF.Exp,
                        bias=bsc[:, r : r + 1],
                        scale=tsc[:, r : r + 1],
                        accum_out=S2[:, half : half + 1],
                    )
                S_ = smB.tile([128, 1], fp32, tag="S")
                nc.vector.tensor_add(S_, S2[:, 0:1], S2[:, 1:2])
                invS = smB.tile([128, 1], fp32, tag="invS")
                nc.vector.reciprocal(invS, S_)
                nc.vector.scalar_tensor_tensor(
                    out=Macc, in0=w_, scalar=invS, in1=Macc, op0=OP.mult, op1=OP.max
                )

        # ---------------- phase 3: fold M across partitions ----------------
        if _PHASES < 3:
            nc.vector.memset(mfold, 0.0)
        with tc.tile_pool(name="tps", bufs=4, space="PSUM") as tps:
            for j in range(P // 128 if _PHASES >= 3 else 0):
                pt = tps.tile([128, 128], bf16, tag="pt")
                nc.tensor.transpose(pt, Macc[:, j * 128 : (j + 1) * 128], ident)
                nc.vector.tensor_reduce(
                    out=mfold[:, j : j + 1], in_=pt, axis=X, op=OP.max
                )
        nc.sync.dma_start(out=m_d[:, :].rearrange("j p -> p j"), in_=mfold)

    _split_excess_waits(nc, mybir, maxw=1)
    return nc


def _inv_sqrt(nc, mybir, pool, nsq, out):
    """out = 1/sqrt(nsq), ACT sqrt + DVE reciprocal + one Newton step.

    Newton on r ~ 1/sqrt(s): r' = r*(1.5 - 0.5*s*r*r) cleans up the
    coarse ScalarE Sqrt table (65536-ULP budget).
    """
    OP = mybir.AluOpType
    AF = mybir.ActivationFunctionType
    shape = list(nsq.shape)
    t = pool.tile(shape, mybir.dt.float32, tag="invsq_t", name="invsq_t")
    nc.scalar.activation(out=t, in_=nsq, func=AF.Sqrt)
    r = pool.tile(shape, mybir.dt.float32, tag="invsq_r", name="invsq_r")
    nc.vector.reciprocal(r, t)
    e = pool.tile(shape, mybir.dt.float32, tag="invsq_e", name="invsq_e")
    nc.vector.tensor_mul(e, r, r)
    nc.vector.tensor_mul(e, e, nsq)
    nc.vector.tensor_scalar(
        out=e, in0=e, scalar1=-0.5, scalar2=1.5, op0=OP.mult, op1=OP.add
    )
    nc.vector.tensor_mul(out, r, e)


def kernel(x, y):
    from concourse.bass_utils import run_bass_kernel_spmd

    x = np.ascontiguousarray(np.asarray(x, dtype=np.float32))
    y = np.ascontiguousarray(np.asarray(y, dtype=np.float32))
    assert x.shape == (N, C, H, W) and y.shape == (N, C, H, W)

    if "nc" not in _cache:
        _cache["nc"] = _build_nc()
    nc = _cache["nc"]

    in_maps = []
    for c in range(NCORES):
        n, h = c // 2, c % 2
        in_maps.append(
            {
                "xh": np.ascontiguousarray(
                    x[n].reshape(C, P)[:, h * HALF : (h + 1) * HALF]
                ),
                "yn": np.ascontiguousarray(y[n].reshape(C, P)),
            }
        )
    res = run_bass_kernel_spmd(nc, in_maps, core_ids=list(range(NCORES)))
    ms = [r["m_out"].reshape(P) for r in res.results]
    cx = np.empty(N, np.float64)
    for n in range(N):
        m = np.maximum(ms[2 * n], ms[2 * n + 1])
        cx[n] = m.astype(np.float64).mean()
    loss = np.mean(-np.log(cx + EPS))
    return np.asarray(loss, dtype=np.float32)

